# revision 15
# baseline (speedup 1.0000x reference)
"""Multi-head attention (B=2, S=2048, D=2048, H=16, causal+RoPE) on 8 trn2
NeuronCores, tensor-parallel over heads (2 heads per core).

Pipeline per core (heads 2c, 2c+1):
  P1: qkv projection in fp32r (11-bit-mantissa fp32 matmul inputs, 4x faster
      than fp32). Q^T/K^T feature-major [dh, t]; V natural [t, dh] cast to
      bf16 at the PSUM drain. RoPE on-chip: rotate-half via partition-strided
      SBUF-SBUF DMA, elementwise combine on gpsimd; attn_scale*sqrt(dh) is
      folded into the per-head q rope tables.
  P2: attention per (head, batch), per causal q-block:
      stats: blocked scores [q, k] (fp32r), per-row max reduced directly
        from PSUM chunks (diagonal chunks masked via one DVE add).
      main: scores recomputed transposed [k, q] (swapped operands), the
        per-q shift -max added inside the matmul group as a K=1 accumulate
        (fp32r rounding of the shift cancels: normalization uses column sums
        of the same shifted exponentials), P^T = Exp straight out of PSUM on
        ACT into bf16. Z = column sums via ones-row matmul accumulation;
        PV matmul in bf16; PV drain multiplies by broadcast 1/Z.
  AllToAll (one per head) redistributes A^T shards so each core owns all
      heads for its 512-token slice.
  P3: out_proj in bf16 for the core's 512 rows; host concatenates rows.

Precision: q/k path fp32r, v/p/out_proj path bf16 -> ~0.45% rel err.
"""
import math

import numpy as np
import ml_dtypes

import concourse.bass as bass
import concourse.mybir as mybir
import concourse.tile as tile
from concourse import bacc
from concourse.bass_utils import run_bass_kernel_spmd

F32 = mybir.dt.float32
F32R = mybir.dt.float32r
BF16 = mybir.dt.bfloat16
AX = mybir.AxisListType.X
EXP = mybir.ActivationFunctionType.Exp

B, S, D = 2, 2048, 2048
H, DH = 16, 128
NC = 8
T = B * S              # 4096 flat tokens
NT = T // 512          # 8 token tiles of 512
ND = D // 128          # 16 contraction tiles
NQT = S // 128         # 16 q-tiles per batch
TOK = T // NC          # 512 tokens per core

LAST_RESULT = None     # BassKernelResults of the most recent run (for tests)


def _round_f32r(a):
    """fp32r rounds matmul inputs to 11 explicit mantissa bits; pre-round on
    host so the device DMA can feed f32r tiles without a cast pass."""
    u = np.ascontiguousarray(a, np.float32).view(np.uint32)
    u = ((u + np.uint32(1 << 11)) >> 12) << 12
    return u.view(np.float32)


def _build(r1=1, r2=1, r3=1, sim_mode=False):
    """Build the SPMD program. r1/r2/r3 repeat phase 1/2/3 bodies for
    phase-attribution benchmarking (1 = normal); sim_mode skips collectives
    so TimelineSim (single-core) can run the program."""
    nc = bacc.Bacc("TRN2", target_bir_lowering=False, debug=False,
                   num_devices=NC)

    xt_d = nc.declare_dram_parameter("xt", [D, T], F32R, isOutput=False)
    wqk_d = nc.declare_dram_parameter("wqk", [D, 512], F32R, isOutput=False)
    wv_d = nc.declare_dram_parameter("wv", [D, 256], F32R, isOutput=False)
    tabs_d = nc.declare_dram_parameter("tabs", [6, 128, S], F32,
                                       isOutput=False)
    masks_d = nc.declare_dram_parameter("cmask", [4, 128, 512], F32,
                                        isOutput=False)
    maskt_d = nc.declare_dram_parameter("cmaskt", [4, 128, 512], F32,
                                        isOutput=False)
    wout_d = nc.declare_dram_parameter("wout", [D, D], BF16, isOutput=False)
    onesr_d = nc.declare_dram_parameter("onesr", [1, 128], F32R, isOutput=False)
    identr_d = nc.declare_dram_parameter("identr", [128, 128], F32R,
                                         isOutput=False)
    o_d = nc.declare_dram_parameter("o", [TOK, D], F32, isOutput=True)

    a2a_in = [nc.dram_tensor(f"a2a_in{h}", [NC, 128, 512], BF16)
              for h in range(2)]
    a2a_out = [nc.dram_tensor(f"a2a_out{h}", [NC, 128, 512], BF16)
               for h in range(2)]

    with tile.TileContext(nc) as tc:
        with tc.tile_pool(name="res", bufs=1) as res:
            # resident across phases
            v_sb = res.tile([128, 32 * 256], BF16)        # [t%128, ttile*256+f]
            at = [[res.tile([128, S], BF16, name=f"at{h}b{b}", tag=f"at{h}{b}")
                   for b in range(B)] for h in range(2)]
            ones_r = res.tile([1, 128], F32R)
            nc.sync.dma_start(ones_r[:], onesr_d[:])
            ones_b = res.tile([128, 1], BF16)
            nc.vector.memset(ones_b[:], 1.0)
            ident_r = res.tile([128, 128], F32R)
            nc.sync.dma_start(ident_r[:], identr_d[:])

            with tc.tile_pool(name="qkt", bufs=1) as qkt:
                qt = [qkt.tile([128, T], F32R, name=f"qt{h}", tag=f"qt{h}")
                      for h in range(2)]
                kt = [qkt.tile([128, T], F32R, name=f"kt{h}", tag=f"kt{h}")
                      for h in range(2)]
                qkres = qt + kt

                # ---------------- P1: projection + rope ----------------
                with tc.tile_pool(name="p1", bufs=1) as p1, \
                     tc.tile_pool(name="ps1", bufs=1, space="PSUM") as ps1:
                    wqk_sb = p1.tile([128, ND, 512], F32R)
                    for g in range(4):
                        nc.sync.dma_start(
                            wqk_sb[:, 4 * g:4 * g + 4, :],
                            wqk_d[512 * g:512 * (g + 1), :].rearrange(
                                "(a p) f -> p a f", p=128))
                    wv_sb = p1.tile([128, ND, 256], F32R)
                    for g in range(4):
                        nc.sync.dma_start(
                            wv_sb[:, 4 * g:4 * g + 4, :],
                            wv_d[512 * g:512 * (g + 1), :].rearrange(
                                "(a p) f -> p a f", p=128))

                    for _ in range(r1):
                        for tt in range(NT):
                            soff = (tt % 4) * 512   # position offset in batch
                            tab = p1.tile([128, 6, 512], F32, tag="tab",
                                          bufs=1)
                            nc.sync.dma_start(
                                tab[:], tabs_d[:, :, soff:soff + 512]
                                .rearrange("c p f -> p c f"))

                            psq = [ps1.tile([128, 512], F32, name=f"psq{f}",
                                            tag=f"psq{f}") for f in range(4)]
                            psv = [ps1.tile([128, 256], F32, name=f"psv{s_}",
                                            tag=f"psv{s_}") for s_ in range(4)]
                            for g in range(4):      # 4 d-tiles per DMA
                                xt = p1.tile([128, 4, 512], F32R, tag="xt",
                                             bufs=2)
                                nc.sync.dma_start(
                                    xt[:],
                                    xt_d[512 * g:512 * (g + 1),
                                         tt * 512:(tt + 1) * 512]
                                    .rearrange("(a p) t -> p a t", p=128))
                                for a in range(4):
                                    dd = 4 * g + a
                                    for f in range(4):
                                        nc.tensor.matmul(
                                            psq[f][:],
                                            wqk_sb[:, dd,
                                                   f * 128:(f + 1) * 128],
                                            xt[:, a, :], start=(dd == 0),
                                            stop=(dd == ND - 1))
                                    for s_ in range(4):
                                        nc.tensor.matmul(
                                            psv[s_][:],
                                            xt[:, a, s_ * 128:(s_ + 1) * 128],
                                            wv_sb[:, dd, :],
                                            start=(dd == 0),
                                            stop=(dd == ND - 1))

                            # V: psum -> resident bf16 (natural [t, f] layout)
                            for s_ in range(4):
                                gti = tt * 4 + s_   # global 128-token tile
                                nc.vector.tensor_copy(
                                    v_sb[:, gti * 256:(gti + 1) * 256],
                                    psv[s_][:])

                            # rope on q (f=0,1) and k (f=2,3); elementwise on
                            # gpsimd (DVE is loaded, Pool is idle)
                            for f in range(4):
                                ci = (2 * f) if f < 2 else 4
                                raw = p1.tile([128, 512], F32, tag="raw",
                                              bufs=2)
                                nc.vector.tensor_copy(raw[:], psq[f][:])
                                rot = p1.tile([128, 512], F32, tag="rot",
                                              bufs=2)
                                nc.sync.dma_start(rot[0:64, :], raw[1:128:2, :])
                                nc.sync.dma_start(rot[64:128, :],
                                                  raw[0:128:2, :])
                                t1 = p1.tile([128, 512], F32, tag="t1", bufs=2)
                                nc.gpsimd.tensor_mul(t1[:], raw[:],
                                                     tab[:, ci, :])
                                nc.gpsimd.tensor_mul(rot[:], rot[:],
                                                     tab[:, ci + 1, :])
                                nc.gpsimd.tensor_add(
                                    qkres[f][:, tt * 512:(tt + 1) * 512],
                                    t1[:], rot[:])

                # ---------------- P2: attention ----------------
                with tc.tile_pool(name="p2", bufs=1) as p2, \
                     tc.tile_pool(name="ps2", bufs=1, space="PSUM") as ps2:
                    mask_sb = p2.tile([128, 4, 512], F32)
                    nc.sync.dma_start(
                        mask_sb[:], masks_d.rearrange("r p f -> p r f"))
                    maskt_sb = p2.tile([128, 4, 512], F32)
                    nc.sync.dma_start(
                        maskt_sb[:], maskt_d.rearrange("r p f -> p r f"))
                    et = p2.tile([128, 16 * 512], BF16)

                    for _ in range(r2):
                        for hh in range(2):
                            for b in range(B):
                                _attn(nc, p2, ps2, qt[hh], kt[hh], v_sb, et,
                                      mask_sb, maskt_sb, at[hh][b], hh, b,
                                      ones_r, ones_b, ident_r)
                            for b in range(B):
                                for qb in range(4):
                                    nc.sync.dma_start(
                                        a2a_in[hh][b * 4 + qb],
                                        at[hh][b][:, qb * 512:(qb + 1) * 512])
                            if not sim_mode:
                                nc.gpsimd.collective_compute(
                                    "AllToAll", mybir.AluOpType.bypass,
                                    ins=[a2a_in[hh][:].opt()],
                                    outs=[a2a_out[hh][:].opt()],
                                    replica_groups=[list(range(NC))])

            # ---------------- P3: out_proj on own 512 rows ----------------
            with tc.tile_pool(name="p3", bufs=1) as p3, \
                 tc.tile_pool(name="ps3", bufs=1, space="PSUM") as ps3:
                atf = p3.tile([128, 16 * 512], BF16)
                for hh in range(2):
                    for j in range(NC):
                        nc.sync.dma_start(
                            atf[:, (2 * j + hh) * 512:(2 * j + hh + 1) * 512],
                            a2a_out[hh][j])
                wout_sb = p3.tile([128, ND, D], BF16)
                for ft in range(ND):
                    nc.sync.dma_start(wout_sb[:, ft, :],
                                      wout_d[ft * 128:(ft + 1) * 128, :])
                ft_order = [2 * j for j in range(8)] + \
                           [2 * j + 1 for j in range(8)]
                for _ in range(r3):
                    for half in range(2):
                        ops = {}
                        for i_ts, ts in enumerate((2 * half, 2 * half + 1)):
                            for e in range(4):
                                ops[(ts, e)] = ps3.tile(
                                    [128, 512], F32, tag=f"op{i_ts}{e}",
                                    bufs=1, name=f"op{i_ts}{e}")
                        for fi, ft in enumerate(ft_order):
                            for ts in (2 * half, 2 * half + 1):
                                for e in range(4):
                                    nc.tensor.matmul(
                                        ops[(ts, e)][:],
                                        atf[:, ft * 512 + ts * 128:
                                            ft * 512 + (ts + 1) * 128],
                                        wout_sb[:, ft, e * 512:(e + 1) * 512],
                                        start=(fi == 0), stop=(fi == ND - 1))
                        for ts in (2 * half, 2 * half + 1):
                            outt = p3.tile([128, D], F32, tag="outt", bufs=2)
                            for e in range(4):
                                nc.vector.tensor_copy(
                                    outt[:, e * 512:(e + 1) * 512],
                                    ops[(ts, e)][:])
                            nc.sync.dma_start(
                                o_d[ts * 128:(ts + 1) * 128, :], outt[:])

    nc.finalize()
    return nc


def _attn(nc, p2, ps2, qth, kth, v_sb, et, mask_sb, maskt_sb, at_bh, hh, b,
          ones_r, ones_b, ident_r):
    """Causal attention for one (head, batch): writes normalized A^T (bf16)
    into at_bh [128, S]. attn_scale*sqrt(dh) is folded into the q rope
    tables so scores arrive pre-scaled. See module docstring."""
    boff = b * S
    nms = p2.tile([128, 16], F32, tag="nms", bufs=2)
    for qb in range(4):
        # ---- stats: per-row -max for the block's 4 q-tiles ----
        for qi in range(4):
            i = 4 * qb + qi
            cm = p2.tile([128, 4], F32, tag="cm", bufs=2)
            for kb in range(qb + 1):
                n = 512 if kb < qb else 128 * (qi + 1)
                sp = ps2.tile([128, 512], F32, tag="sps1", bufs=2)
                nc.tensor.matmul(
                    sp[:, :n],
                    qth[:, boff + i * 128:boff + (i + 1) * 128],
                    kth[:, boff + kb * 512:boff + kb * 512 + n],
                    start=True, stop=True)
                if kb == qb:    # diagonal chunk: mask, then reduce
                    sdiag = p2.tile([128, 512], F32, tag="sdiag", bufs=2)
                    nc.vector.tensor_add(sdiag[:, :n], sp[:, :n],
                                         mask_sb[:, qi, :n])
                    nc.vector.reduce_max(out=cm[:, kb:kb + 1],
                                         in_=sdiag[:, :n], axis=AX)
                else:
                    nc.vector.reduce_max(out=cm[:, kb:kb + 1],
                                         in_=sp[:, :n], axis=AX)
            nc.vector.reduce_max(out=nms[:, i:i + 1], in_=cm[:, :qb + 1],
                                 axis=AX, negate=True)

        # shift row for the block, rounded to f32r (the rounding error is a
        # per-column constant that cancels against Z below)
        nmr = p2.tile([128, 4], F32R, tag="nmr", bufs=2)
        nc.vector.tensor_copy(nmr[:], nms[:, 4 * qb:4 * qb + 4])
        tps = ps2.tile([4, 128], F32, tag="tps", bufs=1)
        nc.tensor.matmul(tps[:], nmr[:], ident_r[:], start=True, stop=True)
        tcol = p2.tile([4, 128], F32R, tag="tcol", bufs=2)
        nc.vector.tensor_copy(tcol[:], tps[:])
        brow = p2.tile([1, 512], F32R, tag="brow", bufs=2)
        nc.gpsimd.dma_start(brow.rearrange("o (q pp) -> o q pp", pp=128),
                            tcol[:])

        # ---- main pass: [k, q] shifted exponentials, Z, PV ----
        nkt = 4 * qb + 4
        zp = ps2.tile([1, 512], F32, tag="zps", bufs=1)
        ap_ = ps2.tile([128, 512], F32, tag="aps", bufs=2)
        for ktile in range(nkt):
            sp2 = ps2.tile([128, 512], F32, tag="sps2", bufs=2)
            nc.tensor.matmul(
                sp2[:],
                kth[:, boff + ktile * 128:boff + (ktile + 1) * 128],
                qth[:, boff + qb * 512:boff + (qb + 1) * 512],
                start=True, stop=False)
            nc.tensor.matmul(sp2[:], ones_r[:], brow[:],
                             start=False, stop=True)
            etc = et[:, ktile * 512:(ktile + 1) * 512]
            rp = ktile - 4 * qb
            if rp >= 0:      # chunk contains the diagonal: mask needed
                tmp = p2.tile([128, 512], F32, tag="tmp", bufs=3)
                nc.vector.tensor_add(tmp[:], sp2[:], maskt_sb[:, rp, :])
                nc.scalar.activation(etc, tmp[:], EXP)
            else:
                nc.scalar.activation(etc, sp2[:], EXP)
            gti = b * 16 + ktile
            nc.tensor.matmul(zp[:], ones_b[:], etc,
                             start=(ktile == 0), stop=(ktile == nkt - 1))
            nc.tensor.matmul(
                ap_[:],
                v_sb[:, gti * 256 + hh * 128:gti * 256 + (hh + 1) * 128],
                etc, start=(ktile == 0), stop=(ktile == nkt - 1))

        rz = p2.tile([1, 512], F32, tag="rz", bufs=2)
        nc.vector.reciprocal(rz[:], zp[:])
        rzb = p2.tile([128, 512], F32, tag="rzb", bufs=2)
        nc.gpsimd.partition_broadcast(rzb[:], rz[0:1, :])
        nc.vector.tensor_mul(at_bh[:, qb * 512:(qb + 1) * 512], ap_[:],
                             rzb[:])


_NC_CACHE = None


def prepare_in_maps(x, w_qkv, w_out, attn_scale):
    x = np.asarray(x, np.float32)
    w_qkv = np.asarray(w_qkv, np.float32)
    w_out = np.asarray(w_out, np.float32)
    attn_scale = np.asarray(attn_scale, np.float32)

    # host-side layout prep (sharding): feature-major activations
    xt = _round_f32r(x.reshape(T, D).T)                       # [D, T]
    # rope tables, feature-major, rotate-half sign folded into sin.
    # q tables are per-head scaled by sqrt(dh)*attn_scale[h] so scores come
    # out of the matmul pre-scaled (k tables unscaled).
    inv = 1.0 / (10000.0 ** (np.arange(0, DH, 2, dtype=np.float32) / DH))
    th = np.outer(inv, np.arange(S, dtype=np.float32))        # [64, S]
    cosT = np.cos(np.concatenate([th, th], 0)).astype(np.float32)
    sinT = np.sin(np.concatenate([th, th], 0)).astype(np.float32)
    sinT[:64] *= -1.0
    # causal diag-block masks, [q, k] and [k, q] orientations
    kk = np.arange(512)[None, :]
    pp = np.arange(128)[:, None]
    masks = np.stack([np.where(kk <= 128 * r + pp, 0.0, -1e9)
                      for r in range(4)]).astype(np.float32)  # [4, 128, 512]
    maskst = np.stack([np.where(128 * r + pp <= kk, 0.0, -1e9)
                       for r in range(4)]).astype(np.float32)
    woutT = np.ascontiguousarray(w_out.T).astype(ml_dtypes.bfloat16)

    in_maps = []
    for c in range(NC):
        h0 = 2 * c
        wq = w_qkv[128 * h0:128 * h0 + 256]                   # both heads' q
        wk = w_qkv[D + 128 * h0:D + 128 * h0 + 256]
        wv = w_qkv[2 * D + 128 * h0:2 * D + 128 * h0 + 256]
        wqk = _round_f32r(np.concatenate([wq, wk], 0).T)      # [D, 512]
        wvT = _round_f32r(wv.T)                               # [D, 256]
        s0 = math.sqrt(DH) * attn_scale[h0]
        s1 = math.sqrt(DH) * attn_scale[h0 + 1]
        tabs = np.stack([cosT * s0, sinT * s0, cosT * s1, sinT * s1,
                         cosT, sinT])                         # [6, 128, S]
        in_maps.append({
            "xt": xt, "wqk": wqk, "wv": wvT, "tabs": tabs,
            "cmask": masks, "cmaskt": maskst, "wout": woutT,
            "onesr": np.ones((1, 128), np.float32),
            "identr": np.eye(128, dtype=np.float32),
        })
    return in_maps


def kernel(x, mask, w_qkv, w_out, attn_scale):
    global _NC_CACHE, LAST_RESULT
    in_maps = prepare_in_maps(x, w_qkv, w_out, attn_scale)
    if _NC_CACHE is None:
        _NC_CACHE = _build()
    res = run_bass_kernel_spmd(_NC_CACHE, in_maps, list(range(NC)))
    LAST_RESULT = res
    rows = np.concatenate([res.results[c]["o"] for c in range(NC)], 0)
    return rows.reshape(B, S, D)
